# revision 22
# baseline (speedup 1.0000x reference)
"""Trainium2 Bass kernel for nn_ANO_VQC_Model (14-qubit VQC, batch 512).

Math: the circuit's state, viewed as a 128x128 matrix M (rows = qubits 0-6,
cols = qubits 7-13), starts as a real rank-1 outer product u v^T (RY layer on
|+>^14 gives a real product state) and each entangling layer acts as
    M' = A0 M B0^T + A1 M B1^T
(only CNOT(6,7) couples rows and cols; it splits into 2 terms via projectors
on qubit 6).  So the state stays factored: L <- [A0 L | A1 L],
R <- [B0 R | B1 R], M = L R^T with rank <= 64 after 6 layers.  Everything is
real f32.  The two requested expectation values are
    e_q = sum( (L^T G_q L) * (R^T R) ),  G_q = Re(H_q) (x) I  (row space).

The row basis is rotated (host-side, folded into the last layer's A matrices)
so that G_0 becomes diagonal: its L^T G_0 L then needs only a per-partition
scale of L instead of a matmul; G_1 is expressed in the same basis.

Sharding: pure data parallel, 64 batch elements per core on 8 cores.
"""

import os
import sys

import numpy as np

for _p in ("/opt/trn_rl_repo", "/root/.axon_site/_ro/trn_rl_repo"):
    if os.path.isdir(_p) and _p not in sys.path:
        sys.path.append(_p)

import concourse.bass as bass
import concourse.mybir as mybir
import concourse.tile as tile
from concourse import bacc
from concourse.bass_utils import run_bass_kernel_spmd

N_CORES = 8
BATCH = 512
BPC = BATCH // N_CORES  # 64
NQ = 14
DEPTH = 6
DA = 128
DB = 128

F32 = mybir.dt.float32
MM_DT = mybir.dt.bfloat16  # matmul input dtype

_nc_cache = {}


# ----------------------------------------------------------------------------
# Host-side preprocessing (input-dependent constant folding)
# ----------------------------------------------------------------------------

def _ry(theta):
    c, s = np.cos(theta / 2), np.sin(theta / 2)
    return np.array([[c, -s], [s, c]], dtype=np.float64)


_CNOT = np.array(
    [[1, 0, 0, 0], [0, 1, 0, 0], [0, 0, 0, 1], [0, 0, 1, 0]], dtype=np.float64
)


def _kron_list(mats):
    out = mats[0]
    for m in mats[1:]:
        out = np.kron(out, m)
    return out


def _cnot_on(n, ctrl):
    mats, q = [], 0
    while q < n:
        if q == ctrl:
            mats.append(_CNOT)
            q += 2
        else:
            mats.append(np.eye(2))
            q += 1
    return _kron_list(mats)


def _layer_mats(theta_k):
    C_evenA = _cnot_on(7, 0) @ _cnot_on(7, 2) @ _cnot_on(7, 4)
    C_oddA = _cnot_on(7, 1) @ _cnot_on(7, 3) @ _cnot_on(7, 5)
    R_A = _kron_list([_ry(theta_k[w]) for w in range(7)])
    C_evenB = _cnot_on(7, 1) @ _cnot_on(7, 3) @ _cnot_on(7, 5)
    C_oddB = _cnot_on(7, 0) @ _cnot_on(7, 2) @ _cnot_on(7, 4)
    R_B = _kron_list([_ry(theta_k[7 + w]) for w in range(7)])
    rows = np.arange(DA)
    P0 = np.diag((rows % 2 == 0).astype(np.float64))
    P1 = np.diag((rows % 2 == 1).astype(np.float64))
    S = np.zeros((DB, DB))
    S[: DB // 2, DB // 2:] = np.eye(DB // 2)
    S[DB // 2:, : DB // 2] = np.eye(DB // 2)
    A0 = R_A @ C_oddA @ P0 @ C_evenA
    A1 = R_A @ C_oddA @ P1 @ C_evenA
    B0 = R_B @ C_oddB @ C_evenB
    B1 = R_B @ C_oddB @ S @ C_evenB
    return A0, A1, B0, B1


def _measure_mats(A, B, D):
    """Re(H_q) expanded to the 128-dim row space, q = 0, 1."""
    NLOC = 8
    rows_t, cols_t = np.tril_indices(NLOC, -1)
    Gs = []
    for q in range(2):
        tri = np.zeros((NLOC, NLOC))
        tri[rows_t, cols_t] = A[q]
        h = tri + np.diag(np.concatenate([D[q][1:], [0.0]]))
        Hr = h + h.T
        if q == 0:
            G = np.kron(Hr, np.eye(16))  # wires 0,1,2 -> row bits 0-2
        else:
            G = np.kron(np.kron(np.eye(2), Hr), np.eye(8))  # wires 1,2,3
        Gs.append(G)
    return Gs


def _host_prep(X, theta, A, B, D):
    X = np.asarray(X, dtype=np.float64)
    theta = np.asarray(theta, dtype=np.float64)
    A = np.asarray(A, dtype=np.float64)
    B = np.asarray(B, dtype=np.float64)
    D = np.asarray(D, dtype=np.float64)
    nb = X.shape[0]
    c, s = np.cos(X / 2), np.sin(X / 2)
    v0 = (c - s) / np.sqrt(2.0)
    v1 = (c + s) / np.sqrt(2.0)

    def kron_side(ws):
        out = np.ones((nb, 1))
        for w in ws:
            pair = np.stack([v0[:, w], v1[:, w]], axis=1)
            out = (out[:, :, None] * pair[:, None, :]).reshape(nb, -1)
        return out

    U = kron_side(range(7))  # (B, 128)
    V = kron_side(range(7, 14))

    G0, G1 = _measure_mats(A, B, D)
    # rotate the row basis so G0 is diagonal: G0 = Hr0 (x) I16,
    # Hr0 = W L W^T  ->  (W (x) I)^T G0 (W (x) I) = diag(repeat(lam, 16))
    NLOC = 8
    rows_t, cols_t = np.tril_indices(NLOC, -1)
    tri = np.zeros((NLOC, NLOC))
    tri[rows_t, cols_t] = A[0]
    h = tri + np.diag(np.concatenate([D[0][1:], [0.0]]))
    Hr0 = h + h.T
    lam8, W0 = np.linalg.eigh(Hr0)
    Wk = np.kron(W0, np.eye(16))  # orthogonal, 128x128
    lam = np.repeat(lam8, 16)  # (128,)
    G1r = Wk.T @ G1 @ Wk  # G1 in the rotated basis (symmetric)

    AT = np.empty((2 * DEPTH, DA, DA))
    BT = np.empty((2 * DEPTH, DB, DB))
    for k in range(DEPTH):
        A0, A1, B0, B1 = _layer_mats(theta[k])
        if k == DEPTH - 1:
            A0 = Wk.T @ A0  # fold the rotation into the last layer
            A1 = Wk.T @ A1
        AT[2 * k + 0] = A0.T  # lhsT layout: out = lhsT.T @ rhs
        AT[2 * k + 1] = A1.T
        BT[2 * k + 0] = B0.T
        BT[2 * k + 1] = B1.T
    # pack per-layer, partition-major for contiguous DMA: (6, 128, 256)
    at_pack = np.ascontiguousarray(
        AT.reshape(DEPTH, 2, DA, DA).transpose(0, 2, 1, 3).reshape(DEPTH, DA, 2 * DA)
    )
    bt_pack = np.ascontiguousarray(
        BT.reshape(DEPTH, 2, DB, DB).transpose(0, 2, 1, 3).reshape(DEPTH, DB, 2 * DB)
    )
    return U, V, at_pack, bt_pack, G1r, lam


# ----------------------------------------------------------------------------
# Device kernel
# ----------------------------------------------------------------------------

def _build_nc():
    nc = bacc.Bacc("TRN2", target_bir_lowering=False, debug=False)

    ut_d = nc.declare_dram_parameter("ut", [DA, BPC], MM_DT, isOutput=False)
    vt_d = nc.declare_dram_parameter("vt", [DB, BPC], MM_DT, isOutput=False)
    at_d = nc.declare_dram_parameter("at", [DEPTH, DA, 2 * DA], MM_DT, isOutput=False)
    bt_d = nc.declare_dram_parameter("bt", [DEPTH, DB, 2 * DB], MM_DT, isOutput=False)
    g_d = nc.declare_dram_parameter("g", [DA, DA], MM_DT, isOutput=False)
    lam_d = nc.declare_dram_parameter("lam", [DA, 1], F32, isOutput=False)
    out_d = nc.declare_dram_parameter("out", [2, BPC], F32, isOutput=True)

    cast_cnt = [0]

    with tile.TileContext(nc) as tc:
        with (
            tc.tile_pool(name="w", bufs=1) as wpool,
            tc.tile_pool(name="state", bufs=1) as spool,
            tc.tile_pool(name="grp", bufs=2) as gpool,
            tc.tile_pool(name="ps", bufs=2, space="PSUM") as pspool,
            tc.tile_pool(name="ps1", bufs=1, space="PSUM") as pspool1,
        ):
            aw = wpool.tile([DA, 2 * DEPTH * DA], MM_DT, tag="aw")
            bw = wpool.tile([DB, 2 * DEPTH * DB], MM_DT, tag="bw")
            gw = wpool.tile([DA, DA], MM_DT, tag="gw")
            ut = wpool.tile([DA, BPC], MM_DT, tag="ut")
            vt = wpool.tile([DB, BPC], MM_DT, tag="vt")
            lam = wpool.tile([DA, 1], F32, tag="lam")
            ones2 = wpool.tile([128, 2], MM_DT, tag="ones2")
            warm = wpool.tile([128, 512], MM_DT, tag="warm")

            # inputs across three DMA queues; first-layer weights first
            nc.sync.dma_start(out=ut[:], in_=ut_d[:, :])
            nc.scalar.dma_start(out=vt[:], in_=vt_d[:, :])
            for k in range(3):
                nc.sync.dma_start(out=aw[:, k * 256:(k + 1) * 256], in_=at_d[k])
                nc.scalar.dma_start(out=bw[:, k * 256:(k + 1) * 256], in_=bt_d[k])
            for k in range(3, DEPTH):
                nc.gpsimd.dma_start(out=aw[:, k * 256:(k + 1) * 256], in_=at_d[k])
                nc.gpsimd.dma_start(out=bw[:, k * 256:(k + 1) * 256], in_=bt_d[k])
            nc.gpsimd.dma_start(out=gw[:], in_=g_d[:, :])
            nc.gpsimd.dma_start(out=lam[:], in_=lam_d[:, :])
            nc.vector.memset(ones2[:], 0.0)
            nc.vector.memset(ones2[0:64, 0:1], 1.0)
            nc.vector.memset(ones2[64:128, 1:2], 1.0)
            nc.vector.memset(warm[:], 0.125)

            # dummy matmuls: ~3.4us of PE activity flips the HAM clock gate
            # to 8/8 before the real layer matmuls arrive
            for _ in range(8):
                wps = pspool.tile([128, 1024], F32, tag="mm2")
                nc.tensor.matmul(
                    wps[:, 0:512], warm[:, 0:128], warm[:],
                    start=True, stop=True,
                )

            Ltmp = spool.tile([DA, 32 * BPC], MM_DT, tag="Ltmp")
            Lbuf = spool.tile([DA, 64 * BPC], MM_DT, tag="Lbuf")
            Rtmp = spool.tile([DB, 32 * BPC], MM_DT, tag="Rtmp")
            Rbuf = spool.tile([DB, 64 * BPC], MM_DT, tag="Rbuf")
            Pbuf = spool.tile([DA, 2 * 64 * BPC], MM_DT, tag="Pbuf")
            SRsb = spool.tile([128, 32 * 64], F32, tag="SRsb")
            esb = spool.tile([2, BPC], F32, tag="esb")

            def cast_out(dst_ap, src_ap):
                # alternate PSUM->SBUF copies between DVE and ACT
                if cast_cnt[0] % 2 == 0:
                    nc.vector.tensor_copy(dst_ap, src_ap)
                else:
                    nc.scalar.copy(out=dst_ap, in_=src_ap)
                cast_cnt[0] += 1

            # ---- layer recursion, b-major columns --------------------------
            def layer(w_tile, cur, dst, k, n_in, after_unit=None):
                nj_in = n_in // BPC
                dstv = dst[:, :2 * n_in].rearrange(
                    "pp (b t j) -> pp b t j", t=2, j=nj_in
                )
                if n_in <= 512:
                    ps = pspool.tile([128, 1024], F32, tag="mm2")
                    for p in range(2):
                        lhsT = w_tile[:, (2 * k + p) * 128:(2 * k + p + 1) * 128]
                        nc.tensor.matmul(
                            ps[:, p * n_in:(p + 1) * n_in], lhsT, cur[:, :n_in],
                            start=True, stop=True,
                        )
                    src = ps[:, :2 * n_in].rearrange(
                        "pp (t b j) -> pp b t j", t=2, b=BPC
                    )
                    cast_out(dstv, src)
                    if after_unit:
                        after_unit()
                else:
                    nb_unit = 1024 // nj_in
                    for p in range(2):
                        lhsT = w_tile[:, (2 * k + p) * 128:(2 * k + p + 1) * 128]
                        for c0 in range(0, n_in, 1024):
                            ps = pspool.tile([128, 1024], F32, tag="mm2")
                            nc.tensor.matmul(
                                ps[:, 0:512], lhsT, cur[:, c0:c0 + 512],
                                start=True, stop=True,
                            )
                            nc.tensor.matmul(
                                ps[:, 512:1024], lhsT, cur[:, c0 + 512:c0 + 1024],
                                start=True, stop=True,
                            )
                            b0 = c0 // nj_in
                            src = ps[:].rearrange("pp (b j) -> pp b j", j=nj_in)
                            cast_out(dstv[:, b0:b0 + nb_unit, p, :], src)
                            if after_unit:
                                after_unit()
                return dst[:, :2 * n_in], 2 * n_in

            # ---- SR_b = R_b^T R_b blocks (16 batches = 8 dual slots) -------
            def emit_sr_block(t, Rfin):
                srg = pspool.tile([128, 512], F32, tag="fin")
                for s_ in range(8):
                    b0 = t * 16 + 2 * s_
                    r0 = Rfin[:, b0 * 64:(b0 + 1) * 64]
                    r1 = Rfin[:, (b0 + 1) * 64:(b0 + 2) * 64]
                    nc.tensor.matmul(
                        srg[0:64, s_ * 64:(s_ + 1) * 64], r0, r0,
                        start=True, stop=True, tile_position=(0, 0),
                    )
                    nc.tensor.matmul(
                        srg[64:128, s_ * 64:(s_ + 1) * 64], r1, r1,
                        start=True, stop=True, tile_position=(0, 64),
                    )
                nc.scalar.copy(out=SRsb[:, t * 512:(t + 1) * 512], in_=srg[:])

            # R recursion first, so SR blocks can interleave into the L phase
            curR, nR = vt[:], BPC
            for k in range(DEPTH):
                dstR = Rtmp if k % 2 == 0 else Rbuf
                curR, nR = layer(bw, curR, dstR, k, nR)
            Rfin = curR

            sr_pending = list(range(4))
            unit_idx = [0]

            def maybe_sr():
                unit_idx[0] += 1
                if unit_idx[0] >= 7 and sr_pending:
                    emit_sr_block(sr_pending.pop(0), Rfin)

            curL, nL = ut[:], BPC
            for k in range(DEPTH):
                dstL = Ltmp if k % 2 == 0 else Lbuf
                curL, nL = layer(aw, curL, dstL, k, nL, after_unit=maybe_sr)
            Lfin = curL
            while sr_pending:
                emit_sr_block(sr_pending.pop(0), Rfin)

            # ---- P: cols (b, q, j); q0 = lam*L (scale), q1 = G1' L ---------
            NL = 64 * BPC  # 4096
            Pview = Pbuf[:].rearrange("p (b q j) -> p b q j", q=2, j=64)
            for b0 in range(0, BPC, 32):
                nc.vector.tensor_scalar_mul(
                    Pview[:, b0:b0 + 32, 0, :],
                    Lfin[:, b0 * 64:(b0 + 32) * 64].rearrange(
                        "p (b j) -> p b j", j=64
                    ),
                    lam[:],
                )
            for c0 in range(0, NL, 1024):
                ps = pspool.tile([128, 1024], F32, tag="mm2")
                nc.tensor.matmul(
                    ps[:, 0:512], gw[:], Lfin[:, c0:c0 + 512],
                    start=True, stop=True,
                )
                nc.tensor.matmul(
                    ps[:, 512:1024], gw[:], Lfin[:, c0 + 512:c0 + 1024],
                    start=True, stop=True,
                )
                b0 = c0 // 64
                src = ps[:].rearrange("pp (b j) -> pp b j", j=64)
                cast_out(Pview[:, b0:b0 + 16, 1, :], src)

            SRr = SRsb[:].rearrange("p (s j) -> p s j", j=64)

            # ---- per-batch quadratic forms: 4 blocks of 8 dual slots -------
            for h in range(2):
                tq0 = gpool.tile([128, 1024], MM_DT, tag="t0")
                tq1 = gpool.tile([128, 1024], MM_DT, tag="t1")
                tq = [tq0, tq1]
                for tt in range(2):
                    t = h * 2 + tt
                    slg = pspool.tile([128, 1024], F32, tag="mm2")
                    for s_ in range(8):
                        b0 = t * 16 + 2 * s_
                        nc.tensor.matmul(
                            slg[0:64, s_ * 128:(s_ + 1) * 128],
                            Lfin[:, b0 * 64:(b0 + 1) * 64],
                            Pbuf[:, b0 * 128:(b0 + 1) * 128],
                            start=True, stop=True, tile_position=(0, 0),
                        )
                        nc.tensor.matmul(
                            slg[64:128, s_ * 128:(s_ + 1) * 128],
                            Lfin[:, (b0 + 1) * 64:(b0 + 2) * 64],
                            Pbuf[:, (b0 + 1) * 128:(b0 + 2) * 128],
                            start=True, stop=True, tile_position=(0, 64),
                        )
                    slg_r = slg[:].rearrange("p (s q j) -> p s q j", q=2, j=64)
                    srsb_r = SRr[:, t * 8:(t + 1) * 8, :]
                    for q in range(2):
                        t_r = tq[q][:, tt * 512:(tt + 1) * 512].rearrange(
                            "p (s j) -> p s j", j=64
                        )
                        nc.vector.tensor_mul(t_r, slg_r[:, :, q, :], srsb_r)
                for q in range(2):
                    zp = pspool1.tile([2, 1024], F32, tag="zpb")
                    nc.tensor.matmul(
                        zp[:, 0:512], ones2[:], tq[q][:, 0:512],
                        start=True, stop=True,
                    )
                    nc.tensor.matmul(
                        zp[:, 512:1024], ones2[:], tq[q][:, 512:1024],
                        start=True, stop=True,
                    )
                    nc.vector.reduce_sum(
                        out=esb[0:2, q * 32 + h * 16:q * 32 + (h + 1) * 16],
                        in_=zp[:].rearrange("p (g j) -> p g j", j=64),
                        axis=mybir.AxisListType.X,
                    )

            nc.sync.dma_start(out=out_d[:, :], in_=esb[:])

    nc.compile()
    return nc


def _get_nc():
    if "nc" not in _nc_cache:
        _nc_cache["nc"] = _build_nc()
    return _nc_cache["nc"]


# ----------------------------------------------------------------------------
# Entry point
# ----------------------------------------------------------------------------

def _decode_out(raw):
    """raw (2, 64): [parity, q*32 + h*16 + tt*8 + slot] -> (64, 2) e[b, q]."""
    e = np.empty((BPC, 2), dtype=np.float32)
    for par in range(2):
        for h in range(2):
            for tt in range(2):
                for s_ in range(8):
                    b = (h * 2 + tt) * 16 + s_ * 2 + par
                    col = h * 16 + tt * 8 + s_
                    e[b, 0] = raw[par, col]
                    e[b, 1] = raw[par, 32 + col]
    return e


def kernel(X, theta, A, B, D, _trace=False):
    U, V, at_pack, bt_pack, G1r, lam = _host_prep(X, theta, A, B, D)
    np_mm = mybir.dt.np(MM_DT)
    at = np.ascontiguousarray(at_pack, dtype=np_mm)
    bt = np.ascontiguousarray(bt_pack, dtype=np_mm)
    g = np.ascontiguousarray(G1r, dtype=np_mm)
    lam_a = np.ascontiguousarray(lam.reshape(DA, 1), dtype=np.float32)
    in_maps = []
    for i in range(N_CORES):
        sl = slice(i * BPC, (i + 1) * BPC)
        in_maps.append(
            {
                "ut": np.ascontiguousarray(U[sl].T, dtype=np_mm),
                "vt": np.ascontiguousarray(V[sl].T, dtype=np_mm),
                "at": at,
                "bt": bt,
                "g": g,
                "lam": lam_a,
            }
        )
    nc = _get_nc()
    kw = {}
    if _trace:
        import shutil
        import tempfile

        shutil.rmtree("/tmp/vqc_prof", ignore_errors=True)
        os.makedirs("/tmp/vqc_prof", exist_ok=True)
        kw["tmpdir"] = tempfile.mkdtemp(dir="/tmp/vqc_prof")
    res = run_bass_kernel_spmd(nc, in_maps, list(range(N_CORES)), trace=_trace, **kw)
    outs = [_decode_out(res.results[i]["out"]) for i in range(N_CORES)]
    full = np.concatenate(outs, axis=0).astype(np.float32)
    if _trace:
        _nc_cache["last_exec_ns"] = res.exec_time_ns
        _nc_cache["last_results"] = res
    return full


# revision 23
# speedup vs baseline: 1.1942x; 1.1942x over previous
"""Trainium2 Bass kernel for nn_ANO_VQC_Model (14-qubit VQC, batch 512).

Math: the circuit's state, viewed as a 128x128 matrix M (rows = qubits 0-6,
cols = qubits 7-13), starts as a real rank-1 outer product u v^T (RY layer on
|+>^14 gives a real product state) and each entangling layer acts as
    M' = A0 M B0^T + A1 M B1^T
(only CNOT(6,7) couples rows and cols; it splits into 2 terms via projectors
on qubit 6).  So the state stays factored: L <- [A0 L | A1 L],
R <- [B0 R | B1 R], M = L R^T with rank <= 64 after 6 layers.  Everything is
real f32.  The two requested expectation values are
    e_q = sum( (L^T G_q L) * (R^T R) ),  G_q = Re(H_q) (x) I  (row space).

The row basis is rotated (host-side, folded into the last layer's A matrices)
so that G_0 becomes diagonal: its L^T G_0 L then needs only a per-partition
scale of L instead of a matmul; G_1 is expressed in the same basis.

Sharding: pure data parallel, 64 batch elements per core on 8 cores.
"""

import os
import sys

import numpy as np

for _p in ("/opt/trn_rl_repo", "/root/.axon_site/_ro/trn_rl_repo"):
    if os.path.isdir(_p) and _p not in sys.path:
        sys.path.append(_p)

import concourse.bass as bass
import concourse.mybir as mybir
import concourse.tile as tile
from concourse import bacc
from concourse.bass_utils import run_bass_kernel_spmd

N_CORES = 8
BATCH = 512
BPC = BATCH // N_CORES  # 64
NQ = 14
DEPTH = 6
DA = 128
DB = 128

F32 = mybir.dt.float32
MM_DT = mybir.dt.bfloat16  # matmul input dtype

_nc_cache = {}


# ----------------------------------------------------------------------------
# Host-side preprocessing (input-dependent constant folding)
# ----------------------------------------------------------------------------

def _ry(theta):
    c, s = np.cos(theta / 2), np.sin(theta / 2)
    return np.array([[c, -s], [s, c]], dtype=np.float64)


_CNOT = np.array(
    [[1, 0, 0, 0], [0, 1, 0, 0], [0, 0, 0, 1], [0, 0, 1, 0]], dtype=np.float64
)


def _kron_list(mats):
    out = mats[0]
    for m in mats[1:]:
        out = np.kron(out, m)
    return out


def _cnot_on(n, ctrl):
    mats, q = [], 0
    while q < n:
        if q == ctrl:
            mats.append(_CNOT)
            q += 2
        else:
            mats.append(np.eye(2))
            q += 1
    return _kron_list(mats)


def _layer_mats(theta_k):
    C_evenA = _cnot_on(7, 0) @ _cnot_on(7, 2) @ _cnot_on(7, 4)
    C_oddA = _cnot_on(7, 1) @ _cnot_on(7, 3) @ _cnot_on(7, 5)
    R_A = _kron_list([_ry(theta_k[w]) for w in range(7)])
    C_evenB = _cnot_on(7, 1) @ _cnot_on(7, 3) @ _cnot_on(7, 5)
    C_oddB = _cnot_on(7, 0) @ _cnot_on(7, 2) @ _cnot_on(7, 4)
    R_B = _kron_list([_ry(theta_k[7 + w]) for w in range(7)])
    rows = np.arange(DA)
    P0 = np.diag((rows % 2 == 0).astype(np.float64))
    P1 = np.diag((rows % 2 == 1).astype(np.float64))
    S = np.zeros((DB, DB))
    S[: DB // 2, DB // 2:] = np.eye(DB // 2)
    S[DB // 2:, : DB // 2] = np.eye(DB // 2)
    A0 = R_A @ C_oddA @ P0 @ C_evenA
    A1 = R_A @ C_oddA @ P1 @ C_evenA
    B0 = R_B @ C_oddB @ C_evenB
    B1 = R_B @ C_oddB @ S @ C_evenB
    return A0, A1, B0, B1


def _measure_mats(A, B, D):
    """Re(H_q) expanded to the 128-dim row space, q = 0, 1."""
    NLOC = 8
    rows_t, cols_t = np.tril_indices(NLOC, -1)
    Gs = []
    for q in range(2):
        tri = np.zeros((NLOC, NLOC))
        tri[rows_t, cols_t] = A[q]
        h = tri + np.diag(np.concatenate([D[q][1:], [0.0]]))
        Hr = h + h.T
        if q == 0:
            G = np.kron(Hr, np.eye(16))  # wires 0,1,2 -> row bits 0-2
        else:
            G = np.kron(np.kron(np.eye(2), Hr), np.eye(8))  # wires 1,2,3
        Gs.append(G)
    return Gs


def _host_prep(X, theta, A, B, D):
    X = np.asarray(X, dtype=np.float64)
    theta = np.asarray(theta, dtype=np.float64)
    A = np.asarray(A, dtype=np.float64)
    B = np.asarray(B, dtype=np.float64)
    D = np.asarray(D, dtype=np.float64)
    nb = X.shape[0]
    c, s = np.cos(X / 2), np.sin(X / 2)
    v0 = (c - s) / np.sqrt(2.0)
    v1 = (c + s) / np.sqrt(2.0)

    def kron_side(ws):
        out = np.ones((nb, 1))
        for w in ws:
            pair = np.stack([v0[:, w], v1[:, w]], axis=1)
            out = (out[:, :, None] * pair[:, None, :]).reshape(nb, -1)
        return out

    U = kron_side(range(7))  # (B, 128)
    V = kron_side(range(7, 14))

    G0, G1 = _measure_mats(A, B, D)
    # rotate the row basis so G0 is diagonal: G0 = Hr0 (x) I16,
    # Hr0 = W L W^T  ->  (W (x) I)^T G0 (W (x) I) = diag(repeat(lam, 16))
    NLOC = 8
    rows_t, cols_t = np.tril_indices(NLOC, -1)
    tri = np.zeros((NLOC, NLOC))
    tri[rows_t, cols_t] = A[0]
    h = tri + np.diag(np.concatenate([D[0][1:], [0.0]]))
    Hr0 = h + h.T
    lam8, W0 = np.linalg.eigh(Hr0)
    Wk = np.kron(W0, np.eye(16))  # orthogonal, 128x128
    lam = np.repeat(lam8, 16)  # (128,)
    G1r = Wk.T @ G1 @ Wk  # G1 in the rotated basis (symmetric)

    AT = np.empty((2 * DEPTH, DA, DA))
    BT = np.empty((2 * DEPTH, DB, DB))
    for k in range(DEPTH):
        A0, A1, B0, B1 = _layer_mats(theta[k])
        if k == DEPTH - 1:
            A0 = Wk.T @ A0  # fold the rotation into the last layer
            A1 = Wk.T @ A1
        AT[2 * k + 0] = A0.T  # lhsT layout: out = lhsT.T @ rhs
        AT[2 * k + 1] = A1.T
        BT[2 * k + 0] = B0.T
        BT[2 * k + 1] = B1.T
    # pack per-layer, partition-major for contiguous DMA: (6, 128, 256)
    at_pack = np.ascontiguousarray(
        AT.reshape(DEPTH, 2, DA, DA).transpose(0, 2, 1, 3).reshape(DEPTH, DA, 2 * DA)
    )
    bt_pack = np.ascontiguousarray(
        BT.reshape(DEPTH, 2, DB, DB).transpose(0, 2, 1, 3).reshape(DEPTH, DB, 2 * DB)
    )
    return U, V, at_pack, bt_pack, G1r, lam


# ----------------------------------------------------------------------------
# Device kernel
# ----------------------------------------------------------------------------

def _build_nc():
    nc = bacc.Bacc("TRN2", target_bir_lowering=False, debug=False)

    ut_d = nc.declare_dram_parameter("ut", [DA, BPC], MM_DT, isOutput=False)
    vt_d = nc.declare_dram_parameter("vt", [DB, BPC], MM_DT, isOutput=False)
    at_d = nc.declare_dram_parameter("at", [DEPTH, DA, 2 * DA], MM_DT, isOutput=False)
    bt_d = nc.declare_dram_parameter("bt", [DEPTH, DB, 2 * DB], MM_DT, isOutput=False)
    g_d = nc.declare_dram_parameter("g", [DA, DA], MM_DT, isOutput=False)
    lam_d = nc.declare_dram_parameter("lam", [DA, 1], F32, isOutput=False)
    out_d = nc.declare_dram_parameter("out", [2, BPC], F32, isOutput=True)

    cast_cnt = [0]

    with tile.TileContext(nc) as tc:
        with (
            tc.tile_pool(name="w", bufs=1) as wpool,
            tc.tile_pool(name="state", bufs=1) as spool,
            tc.tile_pool(name="grp", bufs=2) as gpool,
            tc.tile_pool(name="ps", bufs=2, space="PSUM") as pspool,
            tc.tile_pool(name="ps1", bufs=1, space="PSUM") as pspool1,
        ):
            aw = wpool.tile([DA, 2 * DEPTH * DA], MM_DT, tag="aw")
            bw = wpool.tile([DB, 2 * DEPTH * DB], MM_DT, tag="bw")
            gw = wpool.tile([DA, DA], MM_DT, tag="gw")
            ut = wpool.tile([DA, BPC], MM_DT, tag="ut")
            vt = wpool.tile([DB, BPC], MM_DT, tag="vt")
            lam = wpool.tile([DA, 1], F32, tag="lam")
            ones2 = wpool.tile([128, 2], MM_DT, tag="ones2")
            warm = wpool.tile([128, 512], MM_DT, tag="warm")

            # inputs across three DMA queues; first-layer weights first
            nc.sync.dma_start(out=ut[:], in_=ut_d[:, :])
            nc.scalar.dma_start(out=vt[:], in_=vt_d[:, :])
            for k in range(DEPTH):
                nc.sync.dma_start(out=aw[:, k * 256:(k + 1) * 256], in_=at_d[k])
                nc.scalar.dma_start(out=bw[:, k * 256:(k + 1) * 256], in_=bt_d[k])
            nc.sync.dma_start(out=gw[:], in_=g_d[:, :])
            nc.scalar.dma_start(out=lam[:], in_=lam_d[:, :])
            nc.vector.memset(ones2[:], 0.0)
            nc.vector.memset(ones2[0:64, 0:1], 1.0)
            nc.vector.memset(ones2[64:128, 1:2], 1.0)
            nc.vector.memset(warm[:], 0.125)

            # dummy matmuls: ~3.4us of PE activity flips the HAM clock gate
            # to 8/8 before the real layer matmuls arrive
            for _ in range(8):
                wps = pspool.tile([128, 1024], F32, tag="mm2")
                nc.tensor.matmul(
                    wps[:, 0:512], warm[:, 0:128], warm[:],
                    start=True, stop=True,
                )

            Ltmp = spool.tile([DA, 32 * BPC], MM_DT, tag="Ltmp")
            Lbuf = spool.tile([DA, 64 * BPC], MM_DT, tag="Lbuf")
            Rtmp = spool.tile([DB, 32 * BPC], MM_DT, tag="Rtmp")
            Rbuf = spool.tile([DB, 64 * BPC], MM_DT, tag="Rbuf")
            Pbuf = spool.tile([DA, 2 * 64 * BPC], MM_DT, tag="Pbuf")
            SRsb = spool.tile([128, 32 * 64], F32, tag="SRsb")
            esb = spool.tile([2, BPC], F32, tag="esb")

            def cast_out(dst_ap, src_ap):
                # alternate PSUM->SBUF copies between DVE and ACT
                if cast_cnt[0] % 2 == 0:
                    nc.vector.tensor_copy(dst_ap, src_ap)
                else:
                    nc.scalar.copy(out=dst_ap, in_=src_ap)
                cast_cnt[0] += 1

            # ---- layer recursion, b-major columns --------------------------
            def layer(w_tile, cur, dst, k, n_in, after_unit=None):
                nj_in = n_in // BPC
                dstv = dst[:, :2 * n_in].rearrange(
                    "pp (b t j) -> pp b t j", t=2, j=nj_in
                )
                if n_in <= 512:
                    ps = pspool.tile([128, 1024], F32, tag="mm2")
                    for p in range(2):
                        lhsT = w_tile[:, (2 * k + p) * 128:(2 * k + p + 1) * 128]
                        nc.tensor.matmul(
                            ps[:, p * n_in:(p + 1) * n_in], lhsT, cur[:, :n_in],
                            start=True, stop=True,
                        )
                    src = ps[:, :2 * n_in].rearrange(
                        "pp (t b j) -> pp b t j", t=2, b=BPC
                    )
                    cast_out(dstv, src)
                    if after_unit:
                        after_unit()
                else:
                    nb_unit = 1024 // nj_in
                    for p in range(2):
                        lhsT = w_tile[:, (2 * k + p) * 128:(2 * k + p + 1) * 128]
                        for c0 in range(0, n_in, 1024):
                            ps = pspool.tile([128, 1024], F32, tag="mm2")
                            nc.tensor.matmul(
                                ps[:, 0:512], lhsT, cur[:, c0:c0 + 512],
                                start=True, stop=True,
                            )
                            nc.tensor.matmul(
                                ps[:, 512:1024], lhsT, cur[:, c0 + 512:c0 + 1024],
                                start=True, stop=True,
                            )
                            b0 = c0 // nj_in
                            src = ps[:].rearrange("pp (b j) -> pp b j", j=nj_in)
                            cast_out(dstv[:, b0:b0 + nb_unit, p, :], src)
                            if after_unit:
                                after_unit()
                return dst[:, :2 * n_in], 2 * n_in

            # ---- SR_b = R_b^T R_b blocks (16 batches = 8 dual slots) -------
            def emit_sr_block(t, Rfin):
                srg = pspool.tile([128, 512], F32, tag="fin")
                for s_ in range(8):
                    b0 = t * 16 + 2 * s_
                    r0 = Rfin[:, b0 * 64:(b0 + 1) * 64]
                    r1 = Rfin[:, (b0 + 1) * 64:(b0 + 2) * 64]
                    nc.tensor.matmul(
                        srg[0:64, s_ * 64:(s_ + 1) * 64], r0, r0,
                        start=True, stop=True, tile_position=(0, 0),
                    )
                    nc.tensor.matmul(
                        srg[64:128, s_ * 64:(s_ + 1) * 64], r1, r1,
                        start=True, stop=True, tile_position=(0, 64),
                    )
                nc.scalar.copy(out=SRsb[:, t * 512:(t + 1) * 512], in_=srg[:])

            curL, nL = ut[:], BPC
            curR, nR = vt[:], BPC
            for k in range(DEPTH):
                dstL = Ltmp if k % 2 == 0 else Lbuf
                dstR = Rtmp if k % 2 == 0 else Rbuf
                curL, nL = layer(aw, curL, dstL, k, nL)
                curR, nR = layer(bw, curR, dstR, k, nR)
            Lfin, Rfin = curL, curR

            for t in range(4):
                emit_sr_block(t, Rfin)

            # ---- P: cols (b, q, j); q0 = lam*L (scale), q1 = G1' L ---------
            NL = 64 * BPC  # 4096
            Pview = Pbuf[:].rearrange("p (b q j) -> p b q j", q=2, j=64)
            for b0 in range(0, BPC, 32):
                nc.vector.tensor_scalar_mul(
                    Pview[:, b0:b0 + 32, 0, :],
                    Lfin[:, b0 * 64:(b0 + 32) * 64].rearrange(
                        "p (b j) -> p b j", j=64
                    ),
                    lam[:],
                )
            for c0 in range(0, NL, 1024):
                ps = pspool.tile([128, 1024], F32, tag="mm2")
                nc.tensor.matmul(
                    ps[:, 0:512], gw[:], Lfin[:, c0:c0 + 512],
                    start=True, stop=True,
                )
                nc.tensor.matmul(
                    ps[:, 512:1024], gw[:], Lfin[:, c0 + 512:c0 + 1024],
                    start=True, stop=True,
                )
                b0 = c0 // 64
                src = ps[:].rearrange("pp (b j) -> pp b j", j=64)
                cast_out(Pview[:, b0:b0 + 16, 1, :], src)

            SRr = SRsb[:].rearrange("p (s j) -> p s j", j=64)

            # ---- per-batch quadratic forms: 4 blocks of 8 dual slots -------
            for h in range(2):
                tq0 = gpool.tile([128, 1024], MM_DT, tag="t0")
                tq1 = gpool.tile([128, 1024], MM_DT, tag="t1")
                tq = [tq0, tq1]
                for tt in range(2):
                    t = h * 2 + tt
                    slg = pspool.tile([128, 1024], F32, tag="mm2")
                    for s_ in range(8):
                        b0 = t * 16 + 2 * s_
                        nc.tensor.matmul(
                            slg[0:64, s_ * 128:(s_ + 1) * 128],
                            Lfin[:, b0 * 64:(b0 + 1) * 64],
                            Pbuf[:, b0 * 128:(b0 + 1) * 128],
                            start=True, stop=True, tile_position=(0, 0),
                        )
                        nc.tensor.matmul(
                            slg[64:128, s_ * 128:(s_ + 1) * 128],
                            Lfin[:, (b0 + 1) * 64:(b0 + 2) * 64],
                            Pbuf[:, (b0 + 1) * 128:(b0 + 2) * 128],
                            start=True, stop=True, tile_position=(0, 64),
                        )
                    slg_r = slg[:].rearrange("p (s q j) -> p s q j", q=2, j=64)
                    srsb_r = SRr[:, t * 8:(t + 1) * 8, :]
                    for q in range(2):
                        t_r = tq[q][:, tt * 512:(tt + 1) * 512].rearrange(
                            "p (s j) -> p s j", j=64
                        )
                        nc.vector.tensor_mul(t_r, slg_r[:, :, q, :], srsb_r)
                for q in range(2):
                    zp = pspool1.tile([2, 1024], F32, tag="zpb")
                    nc.tensor.matmul(
                        zp[:, 0:512], ones2[:], tq[q][:, 0:512],
                        start=True, stop=True,
                    )
                    nc.tensor.matmul(
                        zp[:, 512:1024], ones2[:], tq[q][:, 512:1024],
                        start=True, stop=True,
                    )
                    nc.vector.reduce_sum(
                        out=esb[0:2, q * 32 + h * 16:q * 32 + (h + 1) * 16],
                        in_=zp[:].rearrange("p (g j) -> p g j", j=64),
                        axis=mybir.AxisListType.X,
                    )

            nc.sync.dma_start(out=out_d[:, :], in_=esb[:])

    nc.compile()
    return nc


def _get_nc():
    if "nc" not in _nc_cache:
        _nc_cache["nc"] = _build_nc()
    return _nc_cache["nc"]


# ----------------------------------------------------------------------------
# Entry point
# ----------------------------------------------------------------------------

def _decode_out(raw):
    """raw (2, 64): [parity, q*32 + h*16 + tt*8 + slot] -> (64, 2) e[b, q]."""
    e = np.empty((BPC, 2), dtype=np.float32)
    for par in range(2):
        for h in range(2):
            for tt in range(2):
                for s_ in range(8):
                    b = (h * 2 + tt) * 16 + s_ * 2 + par
                    col = h * 16 + tt * 8 + s_
                    e[b, 0] = raw[par, col]
                    e[b, 1] = raw[par, 32 + col]
    return e


def kernel(X, theta, A, B, D, _trace=False):
    U, V, at_pack, bt_pack, G1r, lam = _host_prep(X, theta, A, B, D)
    np_mm = mybir.dt.np(MM_DT)
    at = np.ascontiguousarray(at_pack, dtype=np_mm)
    bt = np.ascontiguousarray(bt_pack, dtype=np_mm)
    g = np.ascontiguousarray(G1r, dtype=np_mm)
    lam_a = np.ascontiguousarray(lam.reshape(DA, 1), dtype=np.float32)
    in_maps = []
    for i in range(N_CORES):
        sl = slice(i * BPC, (i + 1) * BPC)
        in_maps.append(
            {
                "ut": np.ascontiguousarray(U[sl].T, dtype=np_mm),
                "vt": np.ascontiguousarray(V[sl].T, dtype=np_mm),
                "at": at,
                "bt": bt,
                "g": g,
                "lam": lam_a,
            }
        )
    nc = _get_nc()
    kw = {}
    if _trace:
        import shutil
        import tempfile

        shutil.rmtree("/tmp/vqc_prof", ignore_errors=True)
        os.makedirs("/tmp/vqc_prof", exist_ok=True)
        kw["tmpdir"] = tempfile.mkdtemp(dir="/tmp/vqc_prof")
    res = run_bass_kernel_spmd(nc, in_maps, list(range(N_CORES)), trace=_trace, **kw)
    outs = [_decode_out(res.results[i]["out"]) for i in range(N_CORES)]
    full = np.concatenate(outs, axis=0).astype(np.float32)
    if _trace:
        _nc_cache["last_exec_ns"] = res.exec_time_ns
        _nc_cache["last_results"] = res
    return full


# revision 24
# speedup vs baseline: 1.6379x; 1.3716x over previous
"""Trainium2 Bass kernel for nn_ANO_VQC_Model (14-qubit VQC, batch 512).

Math: the circuit's state, viewed as a 128x128 matrix M (rows = qubits 0-6,
cols = qubits 7-13), starts as a real rank-1 outer product u v^T (RY layer on
|+>^14 gives a real product state) and each entangling layer acts as
    M' = A0 M B0^T + A1 M B1^T
(only CNOT(6,7) couples rows and cols; it splits into 2 terms via projectors
on qubit 6).  So the state stays factored: L <- [A0 L | A1 L],
R <- [B0 R | B1 R], M = L R^T with rank <= 64 after 6 layers.  Everything is
real f32.  The two requested expectation values are
    e_q = sum( (L^T G_q L) * (R^T R) ),  G_q = Re(H_q) (x) I  (row space).

The row basis is rotated (host-side, folded into the last layer's A matrices)
so that G_0 becomes diagonal: its L^T G_0 L then needs only a per-partition
scale of L instead of a matmul; G_1 is expressed in the same basis.

Sharding: pure data parallel, 64 batch elements per core on 8 cores.
"""

import os
import sys

import numpy as np

for _p in ("/opt/trn_rl_repo", "/root/.axon_site/_ro/trn_rl_repo"):
    if os.path.isdir(_p) and _p not in sys.path:
        sys.path.append(_p)

import concourse.bass as bass
import concourse.mybir as mybir
import concourse.tile as tile
from concourse import bacc
from concourse.bass_utils import run_bass_kernel_spmd

N_CORES = 8
BATCH = 512
BPC = BATCH // N_CORES  # 64
NQ = 14
DEPTH = 6
DA = 128
DB = 128

F32 = mybir.dt.float32
MM_DT = mybir.dt.bfloat16  # matmul input dtype

_nc_cache = {}


# ----------------------------------------------------------------------------
# Host-side preprocessing (input-dependent constant folding)
# ----------------------------------------------------------------------------

def _ry(theta):
    c, s = np.cos(theta / 2), np.sin(theta / 2)
    return np.array([[c, -s], [s, c]], dtype=np.float64)


_CNOT = np.array(
    [[1, 0, 0, 0], [0, 1, 0, 0], [0, 0, 0, 1], [0, 0, 1, 0]], dtype=np.float64
)


def _kron_list(mats):
    out = mats[0]
    for m in mats[1:]:
        out = np.kron(out, m)
    return out


def _cnot_on(n, ctrl):
    mats, q = [], 0
    while q < n:
        if q == ctrl:
            mats.append(_CNOT)
            q += 2
        else:
            mats.append(np.eye(2))
            q += 1
    return _kron_list(mats)


def _layer_mats(theta_k):
    C_evenA = _cnot_on(7, 0) @ _cnot_on(7, 2) @ _cnot_on(7, 4)
    C_oddA = _cnot_on(7, 1) @ _cnot_on(7, 3) @ _cnot_on(7, 5)
    R_A = _kron_list([_ry(theta_k[w]) for w in range(7)])
    C_evenB = _cnot_on(7, 1) @ _cnot_on(7, 3) @ _cnot_on(7, 5)
    C_oddB = _cnot_on(7, 0) @ _cnot_on(7, 2) @ _cnot_on(7, 4)
    R_B = _kron_list([_ry(theta_k[7 + w]) for w in range(7)])
    rows = np.arange(DA)
    P0 = np.diag((rows % 2 == 0).astype(np.float64))
    P1 = np.diag((rows % 2 == 1).astype(np.float64))
    S = np.zeros((DB, DB))
    S[: DB // 2, DB // 2:] = np.eye(DB // 2)
    S[DB // 2:, : DB // 2] = np.eye(DB // 2)
    A0 = R_A @ C_oddA @ P0 @ C_evenA
    A1 = R_A @ C_oddA @ P1 @ C_evenA
    B0 = R_B @ C_oddB @ C_evenB
    B1 = R_B @ C_oddB @ S @ C_evenB
    return A0, A1, B0, B1


def _measure_mats(A, B, D):
    """Re(H_q) expanded to the 128-dim row space, q = 0, 1."""
    NLOC = 8
    rows_t, cols_t = np.tril_indices(NLOC, -1)
    Gs = []
    for q in range(2):
        tri = np.zeros((NLOC, NLOC))
        tri[rows_t, cols_t] = A[q]
        h = tri + np.diag(np.concatenate([D[q][1:], [0.0]]))
        Hr = h + h.T
        if q == 0:
            G = np.kron(Hr, np.eye(16))  # wires 0,1,2 -> row bits 0-2
        else:
            G = np.kron(np.kron(np.eye(2), Hr), np.eye(8))  # wires 1,2,3
        Gs.append(G)
    return Gs


def _host_prep(X, theta, A, B, D):
    X = np.asarray(X, dtype=np.float64)
    theta = np.asarray(theta, dtype=np.float64)
    A = np.asarray(A, dtype=np.float64)
    B = np.asarray(B, dtype=np.float64)
    D = np.asarray(D, dtype=np.float64)
    nb = X.shape[0]
    c, s = np.cos(X / 2), np.sin(X / 2)
    v0 = (c - s) / np.sqrt(2.0)
    v1 = (c + s) / np.sqrt(2.0)

    def kron_side(ws):
        out = np.ones((nb, 1))
        for w in ws:
            pair = np.stack([v0[:, w], v1[:, w]], axis=1)
            out = (out[:, :, None] * pair[:, None, :]).reshape(nb, -1)
        return out

    U = kron_side(range(7))  # (B, 128)
    V = kron_side(range(7, 14))

    G0, G1 = _measure_mats(A, B, D)
    # rotate the row basis so G0 is diagonal: G0 = Hr0 (x) I16,
    # Hr0 = W L W^T  ->  (W (x) I)^T G0 (W (x) I) = diag(repeat(lam, 16))
    NLOC = 8
    rows_t, cols_t = np.tril_indices(NLOC, -1)
    tri = np.zeros((NLOC, NLOC))
    tri[rows_t, cols_t] = A[0]
    h = tri + np.diag(np.concatenate([D[0][1:], [0.0]]))
    Hr0 = h + h.T
    lam8, W0 = np.linalg.eigh(Hr0)
    Wk = np.kron(W0, np.eye(16))  # orthogonal, 128x128
    lam = np.repeat(lam8, 16)  # (128,)
    G1r = Wk.T @ G1 @ Wk  # G1 in the rotated basis (symmetric)

    As, Bs = [], []
    for k in range(DEPTH):
        A0, A1, B0, B1 = _layer_mats(theta[k])
        As.append((A0, A1))
        Bs.append((B0, B1))
    # fold 3 circuit layers per device stage: stage s, term c = p2*4+p1*2+p0
    # applies  A_{3s+2,p2} A_{3s+1,p1} A_{3s,p0};  the W rotation folds into
    # the last stage of the L side.
    FA = np.empty((2, 8, DA, DA))
    FB = np.empty((2, 8, DB, DB))
    for s in range(2):
        for c in range(8):
            p0, p1, p2 = c & 1, (c >> 1) & 1, (c >> 2) & 1
            Fa = As[3 * s + 2][p2] @ As[3 * s + 1][p1] @ As[3 * s][p0]
            Fb = Bs[3 * s + 2][p2] @ Bs[3 * s + 1][p1] @ Bs[3 * s][p0]
            if s == 1:
                Fa = Wk.T @ Fa
            FA[s, c] = Fa.T  # lhsT layout: out = lhsT.T @ rhs
            FB[s, c] = Fb.T
    # pack per-stage, partition-major for contiguous DMA: (2, 128, 8*128)
    at_pack = np.ascontiguousarray(FA.transpose(0, 2, 1, 3).reshape(2, DA, 8 * DA))
    bt_pack = np.ascontiguousarray(FB.transpose(0, 2, 1, 3).reshape(2, DB, 8 * DB))
    return U, V, at_pack, bt_pack, G1r, lam


# ----------------------------------------------------------------------------
# Device kernel
# ----------------------------------------------------------------------------

def _build_nc():
    nc = bacc.Bacc("TRN2", target_bir_lowering=False, debug=False)

    ut_d = nc.declare_dram_parameter("ut", [DA, BPC], MM_DT, isOutput=False)
    vt_d = nc.declare_dram_parameter("vt", [DB, BPC], MM_DT, isOutput=False)
    at_d = nc.declare_dram_parameter("at", [2, DA, 8 * DA], MM_DT, isOutput=False)
    bt_d = nc.declare_dram_parameter("bt", [2, DB, 8 * DB], MM_DT, isOutput=False)
    g_d = nc.declare_dram_parameter("g", [DA, DA], MM_DT, isOutput=False)
    lam_d = nc.declare_dram_parameter("lam", [DA, 1], F32, isOutput=False)
    out_d = nc.declare_dram_parameter("out", [2, BPC], F32, isOutput=True)

    cast_cnt = [0]

    with tile.TileContext(nc) as tc:
        with (
            tc.tile_pool(name="w", bufs=1) as wpool,
            tc.tile_pool(name="state", bufs=1) as spool,
            tc.tile_pool(name="grp", bufs=2) as gpool,
            tc.tile_pool(name="ps", bufs=2, space="PSUM") as pspool,
            tc.tile_pool(name="ps1", bufs=1, space="PSUM") as pspool1,
        ):
            aw = wpool.tile([DA, 16 * DA], MM_DT, tag="aw")
            bw = wpool.tile([DB, 16 * DB], MM_DT, tag="bw")
            gw = wpool.tile([DA, DA], MM_DT, tag="gw")
            ut = wpool.tile([DA, BPC], MM_DT, tag="ut")
            vt = wpool.tile([DB, BPC], MM_DT, tag="vt")
            lam = wpool.tile([DA, 1], F32, tag="lam")
            ones2 = wpool.tile([128, 2], MM_DT, tag="ones2")
            warm = wpool.tile([128, 512], MM_DT, tag="warm")

            # inputs across three DMA queues; first-layer weights first
            nc.sync.dma_start(out=ut[:], in_=ut_d[:, :])
            nc.scalar.dma_start(out=vt[:], in_=vt_d[:, :])
            for s in range(2):
                nc.sync.dma_start(
                    out=aw[:, s * 1024:(s + 1) * 1024], in_=at_d[s]
                )
                nc.scalar.dma_start(
                    out=bw[:, s * 1024:(s + 1) * 1024], in_=bt_d[s]
                )
            nc.sync.dma_start(out=gw[:], in_=g_d[:, :])
            nc.scalar.dma_start(out=lam[:], in_=lam_d[:, :])
            nc.vector.memset(ones2[:], 0.0)
            nc.vector.memset(ones2[0:64, 0:1], 1.0)
            nc.vector.memset(ones2[64:128, 1:2], 1.0)
            nc.vector.memset(warm[:], 0.125)

            # dummy matmuls: ~3.4us of PE activity flips the HAM clock gate
            # to 8/8 before the real layer matmuls arrive
            for _ in range(8):
                wps = pspool.tile([128, 1024], F32, tag="mm2")
                nc.tensor.matmul(
                    wps[:, 0:512], warm[:, 0:128], warm[:],
                    start=True, stop=True,
                )

            Ltmp = spool.tile([DA, 512], MM_DT, tag="Ltmp")
            Lbuf = spool.tile([DA, 64 * BPC], MM_DT, tag="Lbuf")
            Rtmp = spool.tile([DB, 512], MM_DT, tag="Rtmp")
            Rbuf = spool.tile([DB, 64 * BPC], MM_DT, tag="Rbuf")
            Pbuf = spool.tile([DA, 2 * 64 * BPC], MM_DT, tag="Pbuf")
            SRsb = spool.tile([128, 32 * 64], F32, tag="SRsb")
            esb = spool.tile([2, BPC], F32, tag="esb")

            def cast_out(dst_ap, src_ap):
                # alternate PSUM->SBUF copies between DVE and ACT
                if cast_cnt[0] % 2 == 0:
                    nc.vector.tensor_copy(dst_ap, src_ap)
                else:
                    nc.scalar.copy(out=dst_ap, in_=src_ap)
                cast_cnt[0] += 1

            def cast_split(dst_ap, src_ap):
                # halve the copy latency: DVE takes the front half, ACT the back
                n = dst_ap.shape[1]
                h = n // 2
                nc.vector.tensor_copy(dst_ap[:, :h], src_ap[:, :h])
                nc.scalar.copy(out=dst_ap[:, h:], in_=src_ap[:, h:])

            # ---- SR_b = R_b^T R_b blocks (16 batches = 8 dual slots) -------
            def emit_sr_block(t, Rfin):
                srg = pspool.tile([128, 512], F32, tag="fin")
                for s_ in range(8):
                    b0 = t * 16 + 2 * s_
                    r0 = Rfin[:, b0 * 64:(b0 + 1) * 64]
                    r1 = Rfin[:, (b0 + 1) * 64:(b0 + 2) * 64]
                    nc.tensor.matmul(
                        srg[0:64, s_ * 64:(s_ + 1) * 64], r0, r0,
                        start=True, stop=True, tile_position=(0, 0),
                    )
                    nc.tensor.matmul(
                        srg[64:128, s_ * 64:(s_ + 1) * 64], r1, r1,
                        start=True, stop=True, tile_position=(0, 64),
                    )
                nc.scalar.copy(out=SRsb[:, t * 512:(t + 1) * 512], in_=srg[:])

            # ---- two folded 3-layer stages, b-major columns ----------------
            # stage 1: L3[:, b*8 + c] = F1_c @ u_b   (8 matmuls, 64 cols each)
            def stage1(w_tile, init_ap, dst):
                ps = pspool.tile([128, 1024], F32, tag="mm2")
                for c in range(8):
                    lhsT = w_tile[:, c * 128:(c + 1) * 128]
                    nc.tensor.matmul(
                        ps[:, c * 64:(c + 1) * 64], lhsT, init_ap,
                        start=True, stop=True,
                    )
                # psum cols (c, b) -> dst cols (b, c)
                src_v = ps[:, :512].rearrange("pp (c b) -> pp b c", c=8)
                dst_v = dst[:, :512].rearrange("pp (b c) -> pp b c", c=8)
                cast_split(dst_v, src_v)

            # stage 2: L6[:, b*64 + cp*8 + c3] = F2_cp @ L3[:, (b, c3)]
            def stage2(w_tile, cur, dst):
                dst_v = dst[:].rearrange("pp (b cp c) -> pp b cp c", cp=8, c=8)
                for u in range(4):  # 2 cp terms per 1024-col unit
                    ps = pspool.tile([128, 1024], F32, tag="mm2")
                    for i in range(2):
                        cp = 2 * u + i
                        lhsT = w_tile[:, (8 + cp) * 128:(9 + cp) * 128]
                        nc.tensor.matmul(
                            ps[:, i * 512:(i + 1) * 512], lhsT, cur[:, :512],
                            start=True, stop=True,
                        )
                    src_v = ps[:].rearrange("pp (cp b c) -> pp b cp c", cp=2, c=8)
                    cast_split(dst_v[:, :, 2 * u:2 * u + 2, :], src_v)

            stage1(aw, ut[:], Ltmp)
            stage1(bw, vt[:], Rtmp)
            stage2(aw, Ltmp, Lbuf)
            stage2(bw, Rtmp, Rbuf)
            Lfin, Rfin = Lbuf[:, :], Rbuf[:, :]

            for t in range(4):
                emit_sr_block(t, Rfin)

            # ---- P: cols (b, q, j); q0 = lam*L (scale), q1 = G1' L ---------
            NL = 64 * BPC  # 4096
            Pview = Pbuf[:].rearrange("p (b q j) -> p b q j", q=2, j=64)
            for b0 in range(0, BPC, 32):
                nc.vector.tensor_scalar_mul(
                    Pview[:, b0:b0 + 32, 0, :],
                    Lfin[:, b0 * 64:(b0 + 32) * 64].rearrange(
                        "p (b j) -> p b j", j=64
                    ),
                    lam[:],
                )
            for c0 in range(0, NL, 1024):
                ps = pspool.tile([128, 1024], F32, tag="mm2")
                nc.tensor.matmul(
                    ps[:, 0:512], gw[:], Lfin[:, c0:c0 + 512],
                    start=True, stop=True,
                )
                nc.tensor.matmul(
                    ps[:, 512:1024], gw[:], Lfin[:, c0 + 512:c0 + 1024],
                    start=True, stop=True,
                )
                b0 = c0 // 64
                src = ps[:].rearrange("pp (b j) -> pp b j", j=64)
                cast_out(Pview[:, b0:b0 + 16, 1, :], src)

            SRr = SRsb[:].rearrange("p (s j) -> p s j", j=64)

            # ---- per-batch quadratic forms: 4 blocks of 8 dual slots -------
            for h in range(2):
                tq0 = gpool.tile([128, 1024], MM_DT, tag="t0")
                tq1 = gpool.tile([128, 1024], MM_DT, tag="t1")
                tq = [tq0, tq1]
                for tt in range(2):
                    t = h * 2 + tt
                    slg = pspool.tile([128, 1024], F32, tag="mm2")
                    for s_ in range(8):
                        b0 = t * 16 + 2 * s_
                        nc.tensor.matmul(
                            slg[0:64, s_ * 128:(s_ + 1) * 128],
                            Lfin[:, b0 * 64:(b0 + 1) * 64],
                            Pbuf[:, b0 * 128:(b0 + 1) * 128],
                            start=True, stop=True, tile_position=(0, 0),
                        )
                        nc.tensor.matmul(
                            slg[64:128, s_ * 128:(s_ + 1) * 128],
                            Lfin[:, (b0 + 1) * 64:(b0 + 2) * 64],
                            Pbuf[:, (b0 + 1) * 128:(b0 + 2) * 128],
                            start=True, stop=True, tile_position=(0, 64),
                        )
                    slg_r = slg[:].rearrange("p (s q j) -> p s q j", q=2, j=64)
                    srsb_r = SRr[:, t * 8:(t + 1) * 8, :]
                    for q in range(2):
                        t_r = tq[q][:, tt * 512:(tt + 1) * 512].rearrange(
                            "p (s j) -> p s j", j=64
                        )
                        nc.vector.tensor_mul(t_r, slg_r[:, :, q, :], srsb_r)
                for q in range(2):
                    zp = pspool1.tile([2, 1024], F32, tag="zpb")
                    nc.tensor.matmul(
                        zp[:, 0:512], ones2[:], tq[q][:, 0:512],
                        start=True, stop=True,
                    )
                    nc.tensor.matmul(
                        zp[:, 512:1024], ones2[:], tq[q][:, 512:1024],
                        start=True, stop=True,
                    )
                    nc.vector.reduce_sum(
                        out=esb[0:2, q * 32 + h * 16:q * 32 + (h + 1) * 16],
                        in_=zp[:].rearrange("p (g j) -> p g j", j=64),
                        axis=mybir.AxisListType.X,
                    )

            nc.sync.dma_start(out=out_d[:, :], in_=esb[:])

    nc.compile()
    return nc


def _get_nc():
    if "nc" not in _nc_cache:
        _nc_cache["nc"] = _build_nc()
    return _nc_cache["nc"]


# ----------------------------------------------------------------------------
# Entry point
# ----------------------------------------------------------------------------

def _decode_out(raw):
    """raw (2, 64): [parity, q*32 + h*16 + tt*8 + slot] -> (64, 2) e[b, q]."""
    e = np.empty((BPC, 2), dtype=np.float32)
    for par in range(2):
        for h in range(2):
            for tt in range(2):
                for s_ in range(8):
                    b = (h * 2 + tt) * 16 + s_ * 2 + par
                    col = h * 16 + tt * 8 + s_
                    e[b, 0] = raw[par, col]
                    e[b, 1] = raw[par, 32 + col]
    return e


def kernel(X, theta, A, B, D, _trace=False):
    U, V, at_pack, bt_pack, G1r, lam = _host_prep(X, theta, A, B, D)
    np_mm = mybir.dt.np(MM_DT)
    at = np.ascontiguousarray(at_pack, dtype=np_mm)
    bt = np.ascontiguousarray(bt_pack, dtype=np_mm)
    g = np.ascontiguousarray(G1r, dtype=np_mm)
    lam_a = np.ascontiguousarray(lam.reshape(DA, 1), dtype=np.float32)
    in_maps = []
    for i in range(N_CORES):
        sl = slice(i * BPC, (i + 1) * BPC)
        in_maps.append(
            {
                "ut": np.ascontiguousarray(U[sl].T, dtype=np_mm),
                "vt": np.ascontiguousarray(V[sl].T, dtype=np_mm),
                "at": at,
                "bt": bt,
                "g": g,
                "lam": lam_a,
            }
        )
    nc = _get_nc()
    kw = {}
    if _trace:
        import shutil
        import tempfile

        shutil.rmtree("/tmp/vqc_prof", ignore_errors=True)
        os.makedirs("/tmp/vqc_prof", exist_ok=True)
        kw["tmpdir"] = tempfile.mkdtemp(dir="/tmp/vqc_prof")
    res = run_bass_kernel_spmd(nc, in_maps, list(range(N_CORES)), trace=_trace, **kw)
    outs = [_decode_out(res.results[i]["out"]) for i in range(N_CORES)]
    full = np.concatenate(outs, axis=0).astype(np.float32)
    if _trace:
        _nc_cache["last_exec_ns"] = res.exec_time_ns
        _nc_cache["last_results"] = res
    return full
